# revision 17
# baseline (speedup 1.0000x reference)
"""Trainium2 Bass kernel for CustomStellarEncoder (GNN message passing).

8 NeuronCores, dst-sharded graph parallelism:
  - Stage 1 (replicated): s = x @ W1.T streamed in [feat, node] layout
    (x.T host-packed 2 halves into 128 partitions); BN1 stats accumulated in
    pass 1, feat = relu(A1*s + B1) materialized in pass 2, PE-transposed and
    written as an fp16 gather table [100000, 128] in each core's DRAM.
  - Stage 1b (per-core): own-shard feat.T [128, 12500] fp32 kept in SBUF,
    recomputed from the core's own x shard (xps input).
  - Stage 2 (dst-sharded): edges sorted by dst, 128-dst blocks, per-block
    128-edge chunks (uniform chunk grid across cores so one SPMD program
    fits all). Per chunk: [P,1] indirect-DMA gather of 128 fp16 feat rows
    (+ ones column), one-hot(dst_local) on DVE, PE matmul accumulating
    [dst, feat|cnt] in PSUM. agg = summed * recip(max(cnt,1)), PE-transposed
    into aggT [128, 12544] fp32 in SBUF.
  - sage.T = WlT.T @ aggT + WrT.T @ featT per 512-col chunk; BN2 stats with
    cross-core AllReduce; out_feat.T = A2*sage + B2 written per shard.

b1 and bl are dropped: both cancel exactly under the following BatchNorm.
Outputs are [feat, node] per shard; host transposes and concatenates.
"""

from contextlib import ExitStack

import numpy as np

import concourse.bass as bass
import concourse.tile as tile
from concourse import bacc, mybir
from concourse.bass_utils import run_bass_kernel_spmd
from concourse.masks import make_identity

N_NODES = 100000
N_EDGES = 1600000
IN_DIM = 48
HID = 128
BN_EPS = 1e-5
NCORES = 8
SHARD = N_NODES // NCORES          # 12500
P = 128
NBLK = (SHARD + P - 1) // P        # 98
NODE_PAD = NBLK * P                # 12544
HALF = N_NODES // 2                # 50000
SHALF = SHARD // 2                 # 6250
F32 = mybir.dt.float32
F16 = mybir.dt.float16
I32 = mybir.dt.int32
AX = mybir.AxisListType
ALU = mybir.AluOpType
ACTF = mybir.ActivationFunctionType


def _host_prep(x, edge_index, W1, Wl, Wr, g1, be1, g2, be2):
    xp = np.ascontiguousarray(np.zeros((P, HALF), np.float32))
    xp[0:IN_DIM] = x[:HALF].T
    xp[64:64 + IN_DIM] = x[HALF:].T

    w1tp = np.zeros((P, P), np.float32)
    w1tp[0:IN_DIM] = W1.T
    w1tp[64:64 + IN_DIM] = W1.T

    wlwr = np.zeros((P, 2 * P), np.float32)
    wlwr[:, 0:P] = Wl.T
    wlwr[:, P:2 * P] = Wr.T

    cvec = np.zeros((P, 4), np.float32)
    cvec[:, 0] = g1
    cvec[:, 1] = be1
    cvec[:, 2] = g2
    cvec[:, 3] = be2

    src = np.asarray(edge_index[0], np.int64)
    dst = np.asarray(edge_index[1], np.int64)
    core_of = dst // SHARD
    order = np.argsort(core_of * N_NODES + dst, kind="stable")
    src_s, dst_s = src[order], dst[order]
    core_starts = np.searchsorted(core_of[order] * N_NODES + dst_s,
                                  np.arange(NCORES) * N_NODES * 2)
    # simpler: recompute per-core starts directly
    core_starts = np.searchsorted(core_of[order], np.arange(NCORES + 1))

    nbc = np.zeros((NCORES, NBLK), np.int64)
    per_core = []
    for c in range(NCORES):
        s, e = int(core_starts[c]), int(core_starts[c + 1])
        dl = dst_s[s:e] - c * SHARD          # local dst, sorted ascending
        nbc[c] = np.bincount(dl // P, minlength=NBLK)
        per_core.append((src_s[s:e], dl))
    cpb = np.maximum(1, (nbc.max(axis=0) + P - 1) // P).astype(np.int64)
    offs = np.concatenate([[0], np.cumsum(cpb)]).astype(np.int64)
    C = int(offs[-1])

    srcv = np.zeros((NCORES, P, C), np.int32)
    dstv = np.full((NCORES, P, C), -1.0, np.float32)
    xps = np.zeros((NCORES, P, SHALF), np.float32)
    for c in range(NCORES):
        s_arr, dl = per_core[c]
        bstart = np.concatenate([[0], np.cumsum(nbc[c])])
        for b in range(NBLK):
            n = int(nbc[c][b])
            if n == 0:
                continue
            e0 = int(bstart[b])
            idx = np.arange(n)
            srcv[c, idx % P, offs[b] + idx // P] = s_arr[e0:e0 + n]
            dstv[c, idx % P, offs[b] + idx // P] = \
                (dl[e0:e0 + n] - b * P).astype(np.float32)
        base = c * SHARD
        xps[c, 0:IN_DIM] = x[base:base + SHALF].T
        xps[c, 64:64 + IN_DIM] = x[base + SHALF:base + SHARD].T
    return xp, w1tp, wlwr, cvec, srcv, dstv, xps, cpb, offs, C


def _build(nc, cpb, offs, C):
    cpbmax = int(cpb.max())
    x16d = nc.dram_tensor("x16", [P, HALF], F16, kind="ExternalInput")
    xpsd = nc.dram_tensor("xps", [P, SHALF], F32, kind="ExternalInput")
    w1d = nc.dram_tensor("w1tp", [P, P], F32, kind="ExternalInput")
    wld = nc.dram_tensor("wlwr", [P, 2 * P], F32, kind="ExternalInput")
    cvd = nc.dram_tensor("cvec", [P, 4], F32, kind="ExternalInput")
    srd = nc.dram_tensor("srcv", [P, C], I32, kind="ExternalInput")
    dsd = nc.dram_tensor("dstv", [P, C], F32, kind="ExternalInput")
    featd = nc.dram_tensor("featT", [P, SHARD], F32, kind="ExternalOutput")
    outfd = nc.dram_tensor("outfT", [P, SHARD], F32, kind="ExternalOutput")

    with tile.TileContext(nc) as tc, ExitStack() as ctx:
        persist = ctx.enter_context(tc.tile_pool(name="persist", bufs=1))
        dram = ctx.enter_context(tc.tile_pool(name="dram", bufs=1, space="DRAM"))
        xpool = ctx.enter_context(tc.tile_pool(name="xpool", bufs=3))
        spool = ctx.enter_context(tc.tile_pool(name="spool", bufs=2))
        wpool = ctx.enter_context(tc.tile_pool(name="wpool", bufs=3))
        gpool = ctx.enter_context(tc.tile_pool(name="gpool", bufs=40))
        ohpool = ctx.enter_context(tc.tile_pool(name="ohpool", bufs=2))
        fpool = ctx.enter_context(tc.tile_pool(name="fpool", bufs=3))
        psum = ctx.enter_context(tc.tile_pool(name="psum", bufs=1, space="PSUM"))

        # table rows are 132 wide: 128 feat cols, col 128 = 1.0 (count column
        # picked up for free by the 129-element gather), cols 129-131 pad.
        TW = 132
        table = dram.tile([N_NODES, TW], F16)
        bn_in = dram.tile([P, 2], F32)
        bn_out = dram.tile([P, 2], F32, addr_space="Shared")

        # ---- constants ----
        w1t = persist.tile([P, P], F32)
        nc.sync.dma_start(w1t[:], w1d[:])
        wlwr_t = persist.tile([P, 2 * P], F32)
        nc.sync.dma_start(wlwr_t[:], wld[:])
        cv = persist.tile([P, 4], F32)
        nc.sync.dma_start(cv[:], cvd[:])
        srct = persist.tile([P, C], I32)
        nc.sync.dma_start(srct[:], srd[:])
        dstt = persist.tile([P, C], F32)
        nc.sync.dma_start(dstt[:], dsd[:])

        id16 = persist.tile([P, P], F16)
        make_identity(nc, id16[:])
        id32 = persist.tile([P, P], F32)
        make_identity(nc, id32[:])
        w1t16 = persist.tile([P, P], F16)
        nc.vector.tensor_copy(w1t16[:], w1t[:])
        iota2 = persist.tile([P, P], F32)
        nc.gpsimd.iota(iota2[:], pattern=[[1, P]], base=0, channel_multiplier=0,
                       allow_small_or_imprecise_dtypes=True)
        iotab = persist.tile([P, cpbmax, P], F32)
        for s in range(cpbmax):
            nc.vector.tensor_copy(iotab[:, s:s + 1, :], iota2[:])

        featT = persist.tile([P, NODE_PAD], F32)
        nc.vector.memset(featT[:, SHARD:NODE_PAD], 0.0)
        aggT = persist.tile([P, NODE_PAD], F32)

        st1 = persist.tile([P, 2 * XCH, 6], F32)

        # ================= Stage 1 pass 1: BN1 stats =================
        for k in range(XCH):
            w = min(512, HALF - 512 * k)
            xt16 = xpool.tile([P, 512], F16, tag="xt16")
            nc.sync.dma_start(xt16[:, :w], x16d[:, 512 * k:512 * k + w])
            for h, (p0, slot) in enumerate(((0, 2 * k), (64, 2 * k + 1))):
                ps = psum.tile([P, 512], F32, space="PSUM", tag="ps1", bufs=3)
                nc.tensor.matmul(ps[:, :w], lhsT=w1t16[p0:p0 + IN_DIM, :],
                                 rhs=xt16[p0:p0 + IN_DIM, :w],
                                 start=True, stop=True)
                nc.vector.bn_stats(st1[:, slot:slot + 1, :], ps[:, :w])

        # ---- BN1 coefficient finalize: A1 = g1*rsqrt(var+eps), B1 = be1-mu*A1
        stat = persist.tile([P, 8], F32)
        nc.vector.bn_aggr(stat[:, 2:4], st1[:])     # mean, var(biased)
        nc.vector.tensor_scalar_add(stat[:, 4:5], stat[:, 3:4], BN_EPS)
        nc.vector.reciprocal(stat[:, 5:6], stat[:, 4:5])
        nc.scalar.sqrt(stat[:, 5:6], stat[:, 5:6])                     # rsqrt
        A1 = persist.tile([P, 1], F32)
        nc.vector.tensor_tensor(A1[:], stat[:, 5:6], cv[:, 0:1], op=ALU.mult)
        B1 = persist.tile([P, 1], F32)
        nc.vector.tensor_tensor(B1[:], stat[:, 2:3], A1[:], op=ALU.mult)
        nc.vector.tensor_tensor(B1[:], cv[:, 1:2], B1[:], op=ALU.subtract)

        # ============ Stage 1 pass 2: materialize fp16 table ============
        for k in range(XCH):
            w = min(512, HALF - 512 * k)
            xt16 = xpool.tile([P, 512], F16, tag="xt16")
            nc.sync.dma_start(xt16[:, :w], x16d[:, 512 * k:512 * k + w])
            for h, p0 in enumerate((0, 64)):
                node0 = 512 * k + (0 if h == 0 else HALF)
                ps = psum.tile([P, 512], F32, space="PSUM", tag="ps1", bufs=3)
                nc.tensor.matmul(ps[:, :w], lhsT=w1t16[p0:p0 + IN_DIM, :],
                                 rhs=xt16[p0:p0 + IN_DIM, :w],
                                 start=True, stop=True)
                f16t = wpool.tile([P, 512], F16, tag="f16t")
                nc.scalar.activation(f16t[:, :w], ps[:, :w], ACTF.Relu,
                                     bias=B1[:, 0:1], scale=A1[:, 0:1])
                for s in range((w + P - 1) // P):
                    ww = min(P, w - P * s)
                    pt = psum.tile([P, P], F16, space="PSUM", tag="pt16",
                                   bufs=1)
                    nc.tensor.matmul(pt[:ww, :], lhsT=f16t[:, P * s:P * s + ww],
                                     rhs=id16[:], is_transpose=True,
                                     start=True, stop=True)
                    subt = wpool.tile([P, TW], F16, tag="subt", bufs=4)
                    nc.vector.memset(subt[:, HID:HID + 1], 1.0)
                    nc.vector.tensor_copy(subt[:ww, 0:HID], pt[:ww, :])
                    eng = nc.scalar if s % 2 else nc.sync
                    eng.dma_start(
                        table[node0 + P * s:node0 + P * s + ww, :],
                        subt[:ww, :])

        # ============ Stage 1b: own-shard featT (fp32, SBUF) ============
        SCH = (SHALF + 511) // 512  # 13
        for k in range(SCH):
            w = min(512, SHALF - 512 * k)
            xt2 = xpool.tile([P, 512], F32, tag="xt")
            nc.sync.dma_start(xt2[:, :w], xpsd[:, 512 * k:512 * k + w])
            for h, p0 in enumerate((0, 64)):
                col0 = 512 * k + (0 if h == 0 else SHALF)
                ps = psum.tile([P, 512], F32, space="PSUM", tag="ps1", bufs=3)
                nc.tensor.matmul(ps[:, :w], lhsT=w1t[p0:p0 + IN_DIM, :],
                                 rhs=xt2[p0:p0 + IN_DIM, :w],
                                 start=True, stop=True)
                nc.scalar.activation(featT[:, col0:col0 + w], ps[:, :w],
                                     ACTF.Relu, bias=B1[:, 0:1],
                                     scale=A1[:, 0:1])
        nc.sync.dma_start(featd[:], featT[:, 0:SHARD])

        # ================= Stage 2: edge aggregation =================
        table_ap = table[:]
        for b in range(NBLK):
            nch = int(cpb[b])
            off = int(offs[b])
            ohb = ohpool.tile([P, cpbmax, P], F16, tag="ohb")
            nc.vector.tensor_tensor(
                ohb[:, :nch, :],
                dstt[:, off:off + nch].to_broadcast([P, nch, P]),
                iotab[:, :nch, :], op=ALU.is_equal)
            bps = psum.tile([P, 132], F32, space="PSUM", tag="bps", bufs=3)
            for j in range(nch):
                gt = gpool.tile([P, 132], F16, tag="gt")
                nc.gpsimd.indirect_dma_start(
                    out=gt[:, 0:HID + 1], out_offset=None, in_=table_ap,
                    in_offset=bass.IndirectOffsetOnAxis(
                        ap=srct[:, off + j:off + j + 1], axis=0))
                nc.tensor.matmul(bps[:, 0:HID + 1], lhsT=ohb[:, j:j + 1, :],
                                 rhs=gt[:, 0:HID + 1],
                                 start=(j == 0), stop=(j == nch - 1))
            rmax = fpool.tile([P, 1], F32, tag="rmax")
            nc.vector.tensor_scalar_max(rmax[:], bps[:, HID:HID + 1], 1.0)
            rec = fpool.tile([P, 1], F32, tag="rec")
            nc.vector.reciprocal(rec[:], rmax[:])
            aggsb = fpool.tile([P, P], F32, tag="aggsb")
            nc.scalar.mul(aggsb[:], bps[:, 0:HID], rec[:, 0:1])
            pt2 = psum.tile([P, P], F32, space="PSUM", tag="pt32", bufs=1)
            nc.tensor.matmul(pt2[:], lhsT=aggsb[:], rhs=id32[:],
                             is_transpose=True, start=True, stop=True)
            nc.scalar.copy(aggT[:, b * P:(b + 1) * P], pt2[:])

        # ================= Stage 3: sage + BN2 =================
        NSCH = (NODE_PAD + 511) // 512  # 25 chunks (last = 256)
        st2 = persist.tile([P, NSCH, 6], F32)
        for k in range(NSCH):
            w = min(512, NODE_PAD - 512 * k)
            ws = min(512, max(0, SHARD - 512 * k))   # stats restricted to 12500
            ps = psum.tile([P, 512], F32, space="PSUM", tag="ps1", bufs=3)
            nc.tensor.matmul(ps[:, :w], lhsT=wlwr_t[:, 0:P],
                             rhs=aggT[:, 512 * k:512 * k + w],
                             start=True, stop=False)
            nc.tensor.matmul(ps[:, :w], lhsT=wlwr_t[:, P:2 * P],
                             rhs=featT[:, 512 * k:512 * k + w],
                             start=False, stop=True)
            if ws > 0:
                nc.vector.bn_stats(st2[:, k:k + 1, :], ps[:, :ws])

        stat2 = persist.tile([P, 8], F32)
        mv2 = persist.tile([P, 2], F32)
        nc.vector.bn_aggr(mv2[:], st2[:])           # core-local mean, var
        stats_sb = persist.tile([P, 2], F32)
        nc.scalar.mul(stats_sb[:, 0:1], mv2[:, 0:1], float(SHARD))  # sum
        # sumsq = (var + mean^2) * SHARD
        nc.vector.tensor_tensor(stats_sb[:, 1:2], mv2[:, 0:1], mv2[:, 0:1],
                                op=ALU.mult)
        nc.vector.tensor_tensor(stats_sb[:, 1:2], mv2[:, 1:2],
                                stats_sb[:, 1:2], op=ALU.add)
        nc.vector.tensor_scalar_mul(stats_sb[:, 1:2], stats_sb[:, 1:2],
                                    float(SHARD))
        nc.sync.dma_start(bn_in[:], stats_sb[:])
        nc.gpsimd.collective_compute(
            "AllReduce", ALU.add, replica_groups=[list(range(NCORES))],
            ins=[bn_in[:]], outs=[bn_out[:]])
        gstat = persist.tile([P, 2], F32)
        nc.sync.dma_start(gstat[:], bn_out[:])

        nc.scalar.mul(stat2[:, 2:3], gstat[:, 0:1], 1.0 / N_NODES)   # mu2
        nc.scalar.mul(stat2[:, 3:4], gstat[:, 1:2], 1.0 / N_NODES)   # E[s^2]
        nc.vector.tensor_tensor(stat2[:, 4:5], stat2[:, 2:3], stat2[:, 2:3],
                                op=ALU.mult)
        nc.vector.tensor_tensor(stat2[:, 4:5], stat2[:, 3:4], stat2[:, 4:5],
                                op=ALU.subtract)
        nc.vector.tensor_scalar_add(stat2[:, 4:5], stat2[:, 4:5], BN_EPS)
        nc.vector.reciprocal(stat2[:, 5:6], stat2[:, 4:5])
        nc.scalar.sqrt(stat2[:, 5:6], stat2[:, 5:6])
        A2 = persist.tile([P, 1], F32)
        nc.vector.tensor_tensor(A2[:], stat2[:, 5:6], cv[:, 2:3], op=ALU.mult)
        B2 = persist.tile([P, 1], F32)
        nc.vector.tensor_tensor(B2[:], stat2[:, 2:3], A2[:], op=ALU.mult)
        nc.vector.tensor_tensor(B2[:], cv[:, 3:4], B2[:], op=ALU.subtract)

        for k in range(NSCH):
            ws = min(512, max(0, SHARD - 512 * k))
            if ws <= 0:
                continue
            ps = psum.tile([P, 512], F32, space="PSUM", tag="ps1", bufs=3)
            nc.tensor.matmul(ps[:, :ws], lhsT=wlwr_t[:, 0:P],
                             rhs=aggT[:, 512 * k:512 * k + ws],
                             start=True, stop=False)
            nc.tensor.matmul(ps[:, :ws], lhsT=wlwr_t[:, P:2 * P],
                             rhs=featT[:, 512 * k:512 * k + ws],
                             start=False, stop=True)
            ot = wpool.tile([P, 512], F32, tag="ot")
            nc.scalar.activation(ot[:, :ws], ps[:, :ws], ACTF.Identity,
                                 bias=B2[:, 0:1], scale=A2[:, 0:1])
            nc.sync.dma_start(outfd[:, 512 * k:512 * k + ws], ot[:, :ws])


XCH = (HALF + 511) // 512


def kernel(**inputs):
    x = np.asarray(inputs["x"], np.float32)
    edge_index = np.asarray(inputs["edge_index"])
    args = [x, edge_index,
            np.asarray(inputs["W1"], np.float32),
            np.asarray(inputs["Wl"], np.float32),
            np.asarray(inputs["Wr"], np.float32),
            np.asarray(inputs["g1"], np.float32),
            np.asarray(inputs["be1"], np.float32),
            np.asarray(inputs["g2"], np.float32),
            np.asarray(inputs["be2"], np.float32)]
    xp, w1tp, wlwr, cvec, srcv, dstv, xps, cpb, offs, C = _host_prep(*args)

    x16 = xp.astype(np.float16)

    nc = bacc.Bacc("TRN2", target_bir_lowering=False, debug=False,
                   num_devices=NCORES)
    _build(nc, cpb, offs, C)
    nc.compile()

    in_maps = []
    for c in range(NCORES):
        in_maps.append({
            "x16": x16, "xps": np.ascontiguousarray(xps[c]),
            "w1tp": w1tp, "wlwr": wlwr, "cvec": cvec,
            "srcv": np.ascontiguousarray(srcv[c]),
            "dstv": np.ascontiguousarray(dstv[c]),
        })
    res = run_bass_kernel_spmd(nc, in_maps, core_ids=list(range(NCORES)))
    feat = np.concatenate(
        [res.results[c]["featT"].T for c in range(NCORES)], axis=0)
    out_feat = np.concatenate(
        [res.results[c]["outfT"].T for c in range(NCORES)], axis=0)
    return (np.ascontiguousarray(feat), np.ascontiguousarray(out_feat))
